# revision 12
# baseline (speedup 1.0000x reference)
"""Bass/Tile TRN2 kernel for nn_BatchGraphAttentionLayer.

Reference computation (per batch b):
    Wh  = h[b] @ W                    # [64, 256]
    s1  = Wh @ a[:256], s2 = Wh @ a[256:]
    e   = leaky_relu(s1[i] + s2[j])   # [64, 64]
    att = softmax over axis i of where(adj[i,j]>0, e, -9e15)
    out = elu(att @ Wh)               # contraction over j
Sharding: data-parallel over batch. 8 cores x 4 batches each.

Design notes (v4):
  - Single-bf16 projection (absmax-rel error ~4e-3 vs 2e-2 gate).
  - The 4 local batches form 2 "pairs" of 128 rows. The projection is
    PHASE-SPLIT: stream W (kept resident in SBUF) + pair-0's h first,
    then pair-1's h. Pair-0's whole attention phase overlaps pair-1's
    projection, so only pair-1's attention chain is exposed as tail.
  - Pair-0's attention PE ops are emitted between pair-1's first and
    later projection slabs so the PE never stalls on the DVE score
    chain.
  - h is pre-split on the host into per-pair contiguous [IN, 128]
    arrays so every slab DMA reads 4KB-contiguous per-partition blocks
    (full 16-engine queue spread).
  - e^T[j,i] = s1[i] + s2[j] per pair with one 2-deep bf16 matmul
    (lhsT = [ones; s2], rhs = [s1; ones], rows built by transposing
    column-packed [128,2] tiles - DVE needs partition offset 0).
  - Masking in ONE op: min(leaky, mask_min), mask_min = +BIG/-9e15.
  - att and Wh cast to bf16 for the final matmul (fp32 PE is 4x slower).
  - elu chains split across DVE (pair 0) / Pool (pair 1).
"""

import os
from contextlib import ExitStack

import ml_dtypes
import numpy as np

import concourse.bass as bass
import concourse.tile as tile
from concourse import bacc, mybir
from concourse.bass_utils import run_bass_kernel_spmd

F32 = mybir.dt.float32
BF16 = mybir.dt.bfloat16

B, N, IN, OUT = 32, 64, 16384, 256
NCORES = 8
BPC = B // NCORES            # batches per core = 4
M = BPC * N                  # local rows = 256
P = 128
NEG = -9e15
BIG = 3.0e38
ALPHA = 0.2

KSUB = IN // P               # 128 k-subtiles of 128
NSUB = 16                    # 16 chunks per slab -> all 16 queue engines
SLABS = [NSUB] * (KSUB // NSUB)
NSLAB = len(SLABS)
GATE = int(os.environ.get("GAT_GATE", "1"))
# pair-1 projection slabs emitted before pair-0's attention PE ops
PRE = int(os.environ.get("GAT_PRE", "2"))

_NC = None
LAST_EXEC_NS = None
LAST_RESULTS = None


def _attention(nc, tc, pools, t, ps_wh, sb_arep, sb_eye, sb_mm, out):
    """Emit the attention chain for pair t. Split in two parts so the
    caller can interleave the PE ops with other projection work:
    part 1 (scores, DVE-heavy) is emitted by score(); part 2 by rest()."""
    whpool, small, attp, ps_smallp, ps_ep, ps_op = pools
    st = {}

    def score():
        wh_m = whpool.tile([P, OUT], F32, tag=f"wh_m{t}", name=f"wh_m{t}")
        wh_b = whpool.tile([P, OUT], BF16, tag=f"wh_b{t}", name=f"wh_b{t}")
        nc.vector.tensor_copy(out=wh_m, in_=ps_wh)
        nc.scalar.copy(out=wh_b, in_=ps_wh)
        sc = small.tile([P, 2], F32, tag=f"sc{t}")
        sca = small.tile([P, 2], BF16, tag=f"sc_a{t}")   # [ones | s2]
        scc = small.tile([P, 2], BF16, tag=f"sc_c{t}")   # [s1 | ones]
        nc.gpsimd.memset(sca[:, 0:1], 1.0)
        nc.gpsimd.memset(scc[:, 1:2], 1.0)
        for q in range(2):
            tq = attp.tile([P, OUT], F32, tag=f"s_tmp{t}{q}")
            nc.vector.tensor_tensor(tq, wh_m,
                                    sb_arep[:, q * OUT:(q + 1) * OUT],
                                    mybir.AluOpType.mult)
            nc.vector.tensor_reduce(sc[:, q:q + 1], tq,
                                    axis=mybir.AxisListType.X,
                                    op=mybir.AluOpType.add)
        nc.vector.tensor_copy(out=scc[:, 0:1], in_=sc[:, 0:1])
        nc.vector.tensor_copy(out=sca[:, 1:2], in_=sc[:, 1:2])
        st.update(wh_b=wh_b, sca=sca, scc=scc)

    def rest():
        pl = ps_smallp.tile([2, P], BF16, tag="ps_l", name=f"ps_l{t}")
        nc.tensor.transpose(pl, st["sca"], sb_eye)
        pr = ps_smallp.tile([2, P], BF16, tag="ps_r", name=f"ps_r{t}")
        nc.tensor.transpose(pr, st["scc"], sb_eye)
        lhs_t = small.tile([2, P], BF16, tag=f"eb_l{t}")
        rhs_t = small.tile([2, P], BF16, tag=f"eb_r{t}")
        nc.vector.tensor_copy(out=lhs_t, in_=pl)
        nc.vector.tensor_copy(out=rhs_t, in_=pr)
        ps_e = ps_ep.tile([P, P], F32, tag="ps_e")
        nc.tensor.matmul(ps_e, lhsT=lhs_t, rhs=rhs_t, start=True, stop=True,
                         skip_group_check=True)
        vb = attp.tile([P, P], F32, tag=f"vb{t}")
        nc.vector.tensor_scalar(vb, ps_e, ALPHA, None, mybir.AluOpType.mult)
        lk = attp.tile([P, P], F32, tag=f"lk{t}")
        nc.vector.tensor_tensor(lk, ps_e, vb, mybir.AluOpType.max)
        lkm = attp.tile([P, P], F32, tag=f"lkm{t}")
        nc.vector.tensor_tensor(lkm, lk, sb_mm, mybir.AluOpType.min)
        pexp = attp.tile([P, P], F32, tag=f"pexp{t}")
        rsum = small.tile([P, 1], F32, tag=f"rsum{t}")
        nc.scalar.activation(pexp, lkm, mybir.ActivationFunctionType.Exp,
                             scale=1.0, accum_out=rsum)
        rinv = small.tile([P, 1], F32, tag=f"rinv{t}")
        nc.vector.reciprocal(rinv, rsum)
        att_b = attp.tile([P, P], BF16, tag=f"att_b{t}")
        nc.vector.tensor_scalar_mul(att_b, pexp, rinv)
        ps_o = ps_op.tile([P, OUT], F32, tag="ps_o")
        nc.tensor.matmul(ps_o, lhsT=att_b, rhs=st["wh_b"], start=True,
                         stop=True, skip_group_check=True)
        # elu(x) = max(x,0)-1 + exp(min(x,0)); pair 1 runs on Pool (which
        # can't read PSUM -> evacuate via ACT first)
        if t == 0:
            veng, osrc = nc.vector, ps_o
        else:
            oc = attp.tile([P, OUT], F32, tag=f"oc{t}")
            nc.scalar.copy(out=oc, in_=ps_o)
            veng, osrc = nc.gpsimd, oc
        m0 = attp.tile([P, OUT], F32, tag=f"m0{t}")
        veng.tensor_scalar_min(m0, osrc, 0.0)
        ex = attp.tile([P, OUT], F32, tag=f"ex{t}")
        nc.scalar.activation(ex, m0, mybir.ActivationFunctionType.Exp)
        rm1 = attp.tile([P, OUT], F32, tag=f"rm1{t}")
        veng.tensor_scalar(rm1, osrc, 0.0, -1.0,
                           mybir.AluOpType.max, mybir.AluOpType.add)
        ot = attp.tile([P, OUT], F32, tag=f"ot{t}")
        veng.tensor_tensor(ot, ex, rm1, mybir.AluOpType.add)
        oeng = nc.sync if t == 0 else nc.scalar
        oeng.dma_start(out[t * P:(t + 1) * P, :], ot)

    return score, rest


def _build_kernel(ctx: ExitStack, tc: tile.TileContext, out, h0, h1,
                  w_pack, arep, maskmin, eye_b):
    nc = tc.nc

    consts = ctx.enter_context(tc.tile_pool(name="consts", bufs=1))
    # per-slab tags live for the whole kernel -> bufs=1
    hpool = ctx.enter_context(tc.tile_pool(name="hslab", bufs=1))
    wpool = ctx.enter_context(tc.tile_pool(name="wslab", bufs=1))
    whpool = ctx.enter_context(tc.tile_pool(name="wh", bufs=1))
    small = ctx.enter_context(tc.tile_pool(name="small", bufs=1))
    attp = ctx.enter_context(tc.tile_pool(name="att", bufs=1))
    ps_accp = ctx.enter_context(tc.tile_pool(name="psacc", bufs=1, space="PSUM"))
    ps_smallp = ctx.enter_context(tc.tile_pool(name="pssmall", bufs=1, space="PSUM"))
    ps_ep = ctx.enter_context(tc.tile_pool(name="pse", bufs=1, space="PSUM"))
    ps_op = ctx.enter_context(tc.tile_pool(name="pso", bufs=1, space="PSUM"))
    pools = (whpool, small, attp, ps_smallp, ps_ep, ps_op)

    # consts on gpsimd/SWDGE (only needed in the attention phases)
    sb_arep = consts.tile([P, 2 * OUT], F32)
    nc.gpsimd.dma_start(sb_arep, arep)
    sb_eye = consts.tile([P, P], BF16)
    nc.gpsimd.dma_start(sb_eye, eye_b)
    sb_mm = consts.tile([P, P], F32)
    nc.gpsimd.dma_start(sb_mm, maskmin)

    # ---- stream issue: phase A = (W, h0) slabs, phase B = h1 slabs ----
    ws = [wpool.tile([P, NSUB, OUT], BF16, tag=f"ws{s}", name=f"ws{s}")
          for s in range(NSLAB)]
    h0s = [hpool.tile([P, NSUB, P], BF16, tag=f"h0s{s}", name=f"h0s{s}")
           for s in range(NSLAB)]
    h1s = [hpool.tile([P, NSUB, P], BF16, tag=f"h1s{s}", name=f"h1s{s}")
           for s in range(NSLAB)]
    worder = list(range(NSLAB))
    worder.remove(0)
    worder.insert(GATE, 0)

    def ksl(s):
        return slice(s * NSUB * P, (s + 1) * NSUB * P)

    for i in range(NSLAB):
        qw = nc.sync if i % 2 == 0 else nc.scalar
        qh = nc.scalar if i % 2 == 0 else nc.sync
        sw = worder[i]
        qw.dma_start(ws[sw][:],
                     w_pack[ksl(sw), :].rearrange("(p c) o -> p c o", p=P))
        qh.dma_start(h0s[i][:],
                     h0[ksl(i), :].rearrange("(p c) m -> p c m", p=P))
    for i in range(NSLAB):
        qh = nc.sync if i % 2 == 0 else nc.scalar
        qh.dma_start(h1s[i][:],
                     h1[ksl(i), :].rearrange("(p c) m -> p c m", p=P))

    # ---- pair-0 projection ----
    ps_wh0 = ps_accp.tile([P, OUT], F32, tag="ps_wh0", name="ps_wh0")
    ps_wh1 = ps_accp.tile([P, OUT], F32, tag="ps_wh1", name="ps_wh1")
    for s in range(NSLAB):
        for c in range(NSUB):
            nc.tensor.matmul(ps_wh0, lhsT=h0s[s][:, c, :], rhs=ws[s][:, c, :],
                             start=(s == 0 and c == 0),
                             stop=(s == NSLAB - 1 and c == NSUB - 1),
                             skip_group_check=True)

    score0, rest0 = _attention(nc, tc, pools, 0, ps_wh0, sb_arep, sb_eye,
                               sb_mm, out)
    score0()                     # DVE score chain for pair 0

    # first PRE slabs of pair-1 projection keep the PE busy while the
    # pair-0 score chain runs on DVE
    for s in range(PRE):
        for c in range(NSUB):
            nc.tensor.matmul(ps_wh1, lhsT=h1s[s][:, c, :], rhs=ws[s][:, c, :],
                             start=(s == 0 and c == 0), stop=False,
                             skip_group_check=True)

    rest0()                      # pair-0 attention PE ops + elu + out DMA

    for s in range(PRE, NSLAB):
        for c in range(NSUB):
            nc.tensor.matmul(ps_wh1, lhsT=h1s[s][:, c, :], rhs=ws[s][:, c, :],
                             start=False,
                             stop=(s == NSLAB - 1 and c == NSUB - 1),
                             skip_group_check=True)

    score1, rest1 = _attention(nc, tc, pools, 1, ps_wh1, sb_arep, sb_eye,
                               sb_mm, out)
    score1()
    rest1()


def _get_nc():
    global _NC
    if _NC is not None:
        return _NC
    nc = bacc.Bacc("TRN2", target_bir_lowering=False, debug=False,
                   num_devices=NCORES, disable_frame_to_traceback=True)
    h0 = nc.dram_tensor("h0", [IN, P], BF16, kind="ExternalInput").ap()
    h1 = nc.dram_tensor("h1", [IN, P], BF16, kind="ExternalInput").ap()
    w_pack = nc.dram_tensor("w_pack", [IN, OUT], BF16,
                            kind="ExternalInput").ap()
    arep = nc.dram_tensor("arep", [P, 2 * OUT], F32, kind="ExternalInput").ap()
    maskmin = nc.dram_tensor("maskmin", [P, P], F32, kind="ExternalInput").ap()
    eye_b = nc.dram_tensor("eye_b", [P, P], BF16, kind="ExternalInput").ap()
    out = nc.dram_tensor("out", [M, OUT], F32, kind="ExternalOutput").ap()
    with tile.TileContext(nc) as tc:
        with ExitStack() as ctx:
            _build_kernel(ctx, tc, out, h0, h1, w_pack, arep, maskmin, eye_b)
    nc.compile()
    _NC = nc
    return nc


def _mask_min(adj: np.ndarray):
    adjb = (np.asarray(adj) > 0)                 # [i, j]
    mm = np.full((P, P), np.float32(NEG), np.float32)
    sel = adjb.T                                 # [j, i]
    mm[:N, :N][sel] = BIG
    mm[N:, N:][sel] = BIG
    return mm


def kernel(h: np.ndarray, adj: np.ndarray, W: np.ndarray, a: np.ndarray
           ) -> np.ndarray:
    global LAST_EXEC_NS, LAST_RESULTS
    h = np.asarray(h, dtype=np.float32)
    W = np.asarray(W, dtype=np.float32)
    a = np.ascontiguousarray(np.asarray(a, dtype=np.float32)).reshape(2 * OUT)
    assert h.shape == (B, N, IN) and W.shape == (IN, OUT)

    nc = _get_nc()
    mm = _mask_min(adj)
    eye_b = np.eye(P, dtype=ml_dtypes.bfloat16)
    w_pack = np.ascontiguousarray(W.astype(ml_dtypes.bfloat16))
    arep = np.ascontiguousarray(np.broadcast_to(a[None, :], (P, 2 * OUT)),
                                dtype=np.float32)

    in_maps = []
    for c in range(NCORES):
        hT = h[c * BPC:(c + 1) * BPC].reshape(M, IN).T.astype(
            ml_dtypes.bfloat16)
        imap = {"h0": np.ascontiguousarray(hT[:, :P]),
                "h1": np.ascontiguousarray(hT[:, P:]),
                "w_pack": w_pack, "arep": arep, "maskmin": mm,
                "eye_b": eye_b}
        in_maps.append(imap)

    trace = os.environ.get("GAT_TRACE", "0") == "1"
    res = run_bass_kernel_spmd(nc, in_maps, list(range(NCORES)), trace=trace)
    LAST_EXEC_NS = res.exec_time_ns
    LAST_RESULTS = res

    out = np.empty((B, N, OUT), np.float32)
    for c in range(NCORES):
        out[c * BPC:(c + 1) * BPC] = res.results[c]["out"].reshape(BPC, N, OUT)
    return out


# revision 14
# speedup vs baseline: 1.1697x; 1.1697x over previous
"""Bass/Tile TRN2 kernel for nn_BatchGraphAttentionLayer.

Reference computation (per batch b):
    Wh  = h[b] @ W                    # [64, 256]
    s1  = Wh @ a[:256], s2 = Wh @ a[256:]
    e   = leaky_relu(s1[i] + s2[j])   # [64, 64]
    att = softmax over axis i of where(adj[i,j]>0, e, -9e15)
    out = elu(att @ Wh)               # contraction over j
Sharding: data-parallel over batch. 8 cores x 4 batches each.

Design notes (v5):
  - Single-bf16 projection (absmax-rel error well under the 2e-2 gate).
  - Scores are FREE: w_pack carries two extra bf16 columns
    w12 = [W@a1 | W@a2] (host-precomputed, exact factorization
    s1 = (h W) a1 = h (W a1)), so the projection's moving operand is
    [W | w12] (258 wide) and s1|s2 accumulate in PSUM cols 256:258.
    This deletes the whole DVE score chain and the arep constant.
  - The 4 local batches form 2 pairs of 128 rows; pair t's attention is
    a [128,128] tile with off-diagonal blocks masked to -9e15 (exactly
    0 after softmax), one [128,128]x[128,256] matmul per pair.
  - e^T[j,i] = s1[i] + s2[j] per pair via one 2-deep bf16 matmul
    (lhsT = [ones; s2], rhs = [s1; ones]; rows built by PE-transposing
    column-packed [128,2] tiles - DVE needs partition offset 0).
  - Masking in ONE op: min(leaky, mask_min), mask_min = +BIG/-9e15.
  - att and Wh cast to bf16 for the final matmuls (fp32 PE is 4x/row).
  - ALL elementwise on DVE/ACT. The Pool engine is ~26x slower than DVE
    for [128,256] tiles (measured 3.8us/op) - only tiny memsets go there.
  - All attention PE ops are emitted after the full projection (PE is
    in-order; anything earlier head-of-line-blocks the projection).
"""

import os
from contextlib import ExitStack

import ml_dtypes
import numpy as np

import concourse.bass as bass
import concourse.tile as tile
from concourse import bacc, mybir
from concourse.bass_utils import run_bass_kernel_spmd

F32 = mybir.dt.float32
BF16 = mybir.dt.bfloat16

B, N, IN, OUT = 32, 64, 16384, 256
NCORES = 8
BPC = B // NCORES            # batches per core = 4
M = BPC * N                  # local rows = 256
P = 128
WCOL = OUT + 2               # [W | w1 | w2]
NEG = -9e15
BIG = 3.0e38
ALPHA = 0.2

KSUB = IN // P               # 128 k-subtiles of 128
# small first slabs so the PE starts early (v1-measured-best layout)
SLABS = [2, 14] + [16] * 7
assert sum(SLABS) == KSUB
SLAB_MAX = max(SLABS)

_NC = None
LAST_EXEC_NS = None
LAST_RESULTS = None


def _build_kernel(ctx: ExitStack, tc: tile.TileContext, out, h_hi,
                  w_pack, maskmin, eye_b):
    nc = tc.nc

    consts = ctx.enter_context(tc.tile_pool(name="consts", bufs=1))
    hpool = ctx.enter_context(tc.tile_pool(name="hslab", bufs=5))
    wpool = ctx.enter_context(tc.tile_pool(name="wslab", bufs=5))
    whpool = ctx.enter_context(tc.tile_pool(name="wh", bufs=1))
    small = ctx.enter_context(tc.tile_pool(name="small", bufs=1))
    attp = ctx.enter_context(tc.tile_pool(name="att", bufs=1))
    ps_accp = ctx.enter_context(tc.tile_pool(name="psacc", bufs=1, space="PSUM"))
    ps_smallp = ctx.enter_context(tc.tile_pool(name="pssmall", bufs=1, space="PSUM"))
    ps_ep = ctx.enter_context(tc.tile_pool(name="pse", bufs=1, space="PSUM"))
    ps_op = ctx.enter_context(tc.tile_pool(name="pso", bufs=1, space="PSUM"))

    # consts via gpsimd/SWDGE (needed only in the attention tail)
    sb_eye = consts.tile([P, P], BF16)
    nc.gpsimd.dma_start(sb_eye, eye_b)
    sb_mm = consts.tile([P, P], F32)
    nc.gpsimd.dma_start(sb_mm, maskmin)

    # ---- phase 1: [Wh | s] = h @ [W | w12] accumulated in PSUM ----
    ps_wh = [ps_accp.tile([P, WCOL], F32, tag=f"ps_wh{t}", name=f"ps_wh{t}")
             for t in range(2)]
    k0 = 0
    for s, nsub in enumerate(SLABS):
        ksl = slice(k0 * P, (k0 + nsub) * P)
        ws = wpool.tile([P, SLAB_MAX, WCOL], BF16, tag="ws")
        weng = nc.sync if s % 2 == 0 else nc.scalar
        weng.dma_start(ws[:, :nsub],
                       w_pack[ksl, :].rearrange("(p c) o -> p c o", p=P))
        hs = hpool.tile([P, SLAB_MAX, M], BF16, tag="hs")
        nc.sync.dma_start(hs[:, :nsub],
                          h_hi[ksl, :].rearrange("(p c) m -> p c m", p=P))
        first = (s == 0)
        last = (s == len(SLABS) - 1)
        for c in range(nsub):
            for t in range(2):
                nc.tensor.matmul(ps_wh[t], lhsT=hs[:, c, t * P:(t + 1) * P],
                                 rhs=ws[:, c, :],
                                 start=(first and c == 0),
                                 stop=(last and c == nsub - 1),
                                 skip_group_check=True)
        k0 += nsub

    # ---- phase 2: per-pair attention ----
    # 2a: pack score columns (from PSUM cols 256:258) as bf16
    #     [ones | s2] / [s1 | ones] and bf16-evacuate Wh (ACT engine)
    wh_b, sca, scc = [], [], []
    for t in range(2):
        wb = whpool.tile([P, OUT], BF16, tag=f"wh_b{t}", name=f"wh_b{t}")
        nc.scalar.copy(out=wb, in_=ps_wh[t][:, :OUT])
        a_ = small.tile([P, 2], BF16, tag=f"sc_a{t}")   # [ones | s2]
        c_ = small.tile([P, 2], BF16, tag=f"sc_c{t}")   # [s1 | ones]
        nc.gpsimd.memset(a_[:, 0:1], 1.0)
        nc.gpsimd.memset(c_[:, 1:2], 1.0)
        nc.vector.tensor_copy(out=c_[:, 0:1], in_=ps_wh[t][:, OUT:OUT + 1])
        nc.vector.tensor_copy(out=a_[:, 1:2], in_=ps_wh[t][:, OUT + 1:OUT + 2])
        wh_b.append(wb)
        sca.append(a_)
        scc.append(c_)

    # 2b: transposes + e-build matmuls for both pairs (PE, in order)
    lhs_t, rhs_t, ps_e = [], [], []
    for t in range(2):
        pl = ps_smallp.tile([2, P], BF16, tag="ps_l", name=f"ps_l{t}")
        nc.tensor.transpose(pl, sca[t], sb_eye)
        pr = ps_smallp.tile([2, P], BF16, tag="ps_r", name=f"ps_r{t}")
        nc.tensor.transpose(pr, scc[t], sb_eye)
        lt = small.tile([2, P], BF16, tag=f"eb_l{t}")
        rt = small.tile([2, P], BF16, tag=f"eb_r{t}")
        nc.vector.tensor_copy(out=lt, in_=pl)
        nc.vector.tensor_copy(out=rt, in_=pr)
        lhs_t.append(lt)
        rhs_t.append(rt)
    for t in range(2):
        pe = ps_ep.tile([P, P], F32, tag=f"ps_e{t}", name=f"ps_e{t}")
        nc.tensor.matmul(pe, lhsT=lhs_t[t], rhs=rhs_t[t], start=True,
                         stop=True, skip_group_check=True)
        ps_e.append(pe)

    # 2c: leaky+mask+softmax (DVE/ACT), final matmul, elu, store
    att_b = []
    for t in range(2):
        vb = attp.tile([P, P], F32, tag=f"vb{t}")
        nc.vector.tensor_scalar(vb, ps_e[t], ALPHA, None,
                                mybir.AluOpType.mult)
        lk = attp.tile([P, P], F32, tag=f"lk{t}")
        nc.vector.tensor_tensor(lk, ps_e[t], vb, mybir.AluOpType.max)
        lkm = attp.tile([P, P], F32, tag=f"lkm{t}")
        nc.vector.tensor_tensor(lkm, lk, sb_mm, mybir.AluOpType.min)
        pexp = attp.tile([P, P], F32, tag=f"pexp{t}")
        rsum = small.tile([P, 1], F32, tag=f"rsum{t}")
        nc.scalar.activation(pexp, lkm, mybir.ActivationFunctionType.Exp,
                             scale=1.0, accum_out=rsum)
        rinv = small.tile([P, 1], F32, tag=f"rinv{t}")
        nc.vector.reciprocal(rinv, rsum)
        ab = attp.tile([P, P], BF16, tag=f"att_b{t}")
        nc.vector.tensor_scalar_mul(ab, pexp, rinv)
        att_b.append(ab)
    ps_o = []
    for t in range(2):
        po = ps_op.tile([P, OUT], F32, tag=f"ps_o{t}", name=f"ps_o{t}")
        nc.tensor.matmul(po, lhsT=att_b[t], rhs=wh_b[t], start=True,
                         stop=True, skip_group_check=True)
        ps_o.append(po)
    for t in range(2):
        m0 = attp.tile([P, OUT], F32, tag=f"m0{t}")
        nc.vector.tensor_scalar_min(m0, ps_o[t], 0.0)
        ex = attp.tile([P, OUT], F32, tag=f"ex{t}")
        nc.scalar.activation(ex, m0, mybir.ActivationFunctionType.Exp)
        rm1 = attp.tile([P, OUT], F32, tag=f"rm1{t}")
        nc.vector.tensor_scalar(rm1, ps_o[t], 0.0, -1.0,
                                mybir.AluOpType.max, mybir.AluOpType.add)
        ot = attp.tile([P, OUT], F32, tag=f"ot{t}")
        nc.vector.tensor_tensor(ot, ex, rm1, mybir.AluOpType.add)
        oeng = nc.sync if t == 0 else nc.scalar
        oeng.dma_start(out[t * P:(t + 1) * P, :], ot)


def _get_nc():
    global _NC
    if _NC is not None:
        return _NC
    nc = bacc.Bacc("TRN2", target_bir_lowering=False, debug=False,
                   num_devices=NCORES, disable_frame_to_traceback=True)
    h_hi = nc.dram_tensor("h_hi", [IN, M], BF16, kind="ExternalInput").ap()
    w_pack = nc.dram_tensor("w_pack", [IN, WCOL], BF16,
                            kind="ExternalInput").ap()
    maskmin = nc.dram_tensor("maskmin", [P, P], F32, kind="ExternalInput").ap()
    eye_b = nc.dram_tensor("eye_b", [P, P], BF16, kind="ExternalInput").ap()
    out = nc.dram_tensor("out", [M, OUT], F32, kind="ExternalOutput").ap()
    with tile.TileContext(nc) as tc:
        with ExitStack() as ctx:
            _build_kernel(ctx, tc, out, h_hi, w_pack, maskmin, eye_b)
    nc.compile()
    _NC = nc
    return nc


def _mask_min(adj: np.ndarray):
    adjb = (np.asarray(adj) > 0)                 # [i, j]
    mm = np.full((P, P), np.float32(NEG), np.float32)
    sel = adjb.T                                 # [j, i]
    mm[:N, :N][sel] = BIG
    mm[N:, N:][sel] = BIG
    return mm


def kernel(h: np.ndarray, adj: np.ndarray, W: np.ndarray, a: np.ndarray
           ) -> np.ndarray:
    global LAST_EXEC_NS, LAST_RESULTS
    h = np.asarray(h, dtype=np.float32)
    W = np.asarray(W, dtype=np.float32)
    a = np.ascontiguousarray(np.asarray(a, dtype=np.float32)).reshape(2 * OUT)
    assert h.shape == (B, N, IN) and W.shape == (IN, OUT)

    nc = _get_nc()
    mm = _mask_min(adj)
    eye_b = np.eye(P, dtype=ml_dtypes.bfloat16)
    # w12 = [W@a1 | W@a2]: exact factorization s = h @ (W @ a_q)
    w12 = (W.astype(np.float64) @ a.astype(np.float64).reshape(2, OUT).T)
    w_pack = np.concatenate(
        [W, w12.astype(np.float32)], axis=1).astype(ml_dtypes.bfloat16)
    w_pack = np.ascontiguousarray(w_pack)

    in_maps = []
    for c in range(NCORES):
        hT = h[c * BPC:(c + 1) * BPC].reshape(M, IN).T
        imap = {"h_hi": np.ascontiguousarray(hT).astype(ml_dtypes.bfloat16),
                "w_pack": w_pack, "maskmin": mm, "eye_b": eye_b}
        in_maps.append(imap)

    trace = os.environ.get("GAT_TRACE", "0") == "1"
    res = run_bass_kernel_spmd(nc, in_maps, list(range(NCORES)), trace=trace)
    LAST_EXEC_NS = res.exec_time_ns
    LAST_RESULTS = res

    out = np.empty((B, N, OUT), np.float32)
    for c in range(NCORES):
        out[c * BPC:(c + 1) * BPC] = res.results[c]["out"].reshape(BPC, N, OUT)
    return out
